# revision 1
# baseline (speedup 1.0000x reference)
"""Trainium2 Bass kernel for a single pre-norm transformer block.

Reference (B=2, T=2048, C=768, H=12, HD=64):
    x = x + causal_attn(LN1(x) @ W_qkv) @ W_attn_proj
    x = x + gelu(LN2(x) @ W_fc) @ W_mlp_proj

Sharding: 8 cores, zero collectives (on-chip allreduce has a ~60-100us
firmware floor + ~49GB/s bus -- far too slow here).  Core c = (batch
b=c//4, rank p=c%4).  Causally balanced interleaved query blocks: core p
owns the four 128-row q-blocks {15-p, 11-p, 7-p, 3-p} of its batch, so
every core's q-slots see the uniform k-extents SLOT_BOUNDS=(16,12,8,4)
chunks -- one SPMD program, no per-core control flow, only ~18% causal
overcompute.  Each core recomputes LN1 + K/V for its whole batch
(weights replicated; that is why this problem is memory-regime).

Layout: feature-major activations [C on partitions, tokens free], so the
stationary matmul operand is always a natural weight tile and no
activation transposes exist anywhere.
  - LN stats: matmul with an all-ones stationary tile sums over features
    AND replicates the sums to all partitions (no partition broadcast).
  - scores S^T[k,q]: lhsT = K fm chunk, rhs = Q fm slot; four k-chunks
    per PSUM bank, exp'd in one ScalarE op (bf16 out), multiplicative
    causal mask only on each slot's final 4-chunk group.
  - AV: lhsT = [V token-major | ones] -> Y rows 0..63 + softmax
    denominator row; normalized via DVE reciprocal + DRAM-roundtrip
    broadcast on gpsimd queues.
  - QKV/attention/MLP matmuls in bf16 (attention residual is ~1.4% of
    output magnitude; MLP error dilutes similarly), LN arithmetic and
    residuals in f32.  Weights are host-pre-tiled into the exact SBUF
    images so every weight load is one large contiguous DMA.
  - kernel() specializes away the LN scale/bias ops when gamma==1 and
    beta==0 (the spec fills) -- checked at runtime on the host.
Measured: norm rel err 1.15e-3 on HW; CoreSim cost model ~263us/core.
"""

import sys

if "/opt/trn_rl_repo" not in sys.path:
    sys.path.insert(0, "/opt/trn_rl_repo")

import numpy as np

import concourse.bass as bass
import concourse.mybir as mybir
from concourse import bacc
import concourse.tile as tile

P = 128
B, T, C, H, HD = 2, 2048, 768, 12, 64
OWN = 512          # query rows owned by each core
NF = C // P        # 6 feature chunks
NQT = T // 512     # 4 column tiles over the 2048 tokens
NKT = T // P       # 16 key chunks
NMO_FC = (4 * C) // P  # 24
SLOT_BOUNDS = (16, 12, 8, 4)   # k-chunks processed per q-slot (128 q rows each)
EPS = 1e-5

f32 = mybir.dt.float32
f32r = mybir.dt.float32r
bf16 = mybir.dt.bfloat16
AFT = mybir.ActivationFunctionType
ALU = mybir.AluOpType

GELU_FUNC = AFT.Gelu  # dev sims patch an erf-gelu into bass_interp for this


def _r(ap):
    """View an f32 AP as float32r for full-rate PE matmuls."""
    return ap.bitcast(f32r)


def build_program(unit_gb=False):
    nc = bacc.Bacc()

    xT = nc.declare_dram_parameter("xT", [C, T], f32, False)[:]
    xq = nc.declare_dram_parameter("xq", [C, OWN], f32, False)[:]
    mask4 = nc.declare_dram_parameter("mask4", [P, 4, 4, P], bf16, False)[:]
    Wq_t = nc.declare_dram_parameter("Wq_t", [NF, P, NF, P], bf16, False)[:]
    Wk_t = nc.declare_dram_parameter("Wk_t", [NF, P, NF, P], bf16, False)[:]
    Wv_t = nc.declare_dram_parameter("Wv_t", [2, P, NF, 384], bf16, False)[:]
    Wap = nc.declare_dram_parameter("Wap", [C, C], bf16, False)[:]
    Wfc_t = nc.declare_dram_parameter("Wfc_t", [NF, P, 4, NF, P], bf16, False)[:]
    Wmp_t = nc.declare_dram_parameter("Wmp_t", [NF, P, NMO_FC, P], bf16, False)[:]
    g1 = nc.declare_dram_parameter("g1", [C], f32, False)[:]
    b1 = nc.declare_dram_parameter("b1", [C], f32, False)[:]
    g2 = nc.declare_dram_parameter("g2", [C], f32, False)[:]
    b2 = nc.declare_dram_parameter("b2", [C], f32, False)[:]
    outT = nc.declare_dram_parameter("outT", [C, OWN], f32, True)[:]

    # feature-chunked DRAM views: feature f = o*128 + p
    xT_r = xT.rearrange("(o p) t -> p o t", p=P)
    xq_r = xq.rearrange("(o p) t -> p o t", p=P)
    WapR = Wap.rearrange("(o p) m -> p o m", p=P)  # [128, 6, 768]

    outT_r = outT.rearrange("(o p) q -> p o q", p=P)

    with tile.TileContext(nc) as tc:
        _body(nc, tc, unit_gb, dict(
            xT_r=xT_r, xq_r=xq_r, Wq_t=Wq_t, Wk_t=Wk_t, Wv_t=Wv_t,
            Wfc_t=Wfc_t, Wmp_t=Wmp_t, WapR=WapR, mask4=mask4, outT_r=outT_r,
            g1=g1, b1=b1, g2=g2, b2=b2,
        ))
    nc.finalize()
    return nc


def _body(nc, tc, unit_gb, d):
    from contextlib import ExitStack

    with ExitStack() as ctx:
        def pool(name, bufs, space="SBUF"):
            return ctx.enter_context(tc.tile_pool(name=name, bufs=bufs, space=space))

        singles = pool("singles", 1)
        xstream2 = pool("xstream2", 8)     # x chunks [P,512] f32
        lnpool = pool("lnpool", 6)         # xln1 resident bf16 [P,T]
        wq_p = pool("wq_p", 3)             # W qkv/fc column tiles
        statp = pool("statp", 4)           # LN stats [P,512] f32
        recp = pool("recp", 2)             # [1,512] f32
        rbp = pool("rbp", 2)               # [64,512] f32
        sqp = pool("sqp", 2)               # f32 scratch [P,512]
        bfp = pool("bfp", 6)               # bf16 stats tiles [P,512]
        ypool = pool("ypool", 6)           # head-pair Y bf16 [128,OWN]
        ytmp = pool("ytmp", 2)             # odd-head staging [64,OWN]
        x2pool = pool("x2pool", 6)         # x2 resident f32
        xlnp2 = pool("xlnp2", 6)           # xln2 f32
        opool = pool("opool", 2)
        dramp = pool("dramp", 2, space="DRAM")
        ps_mm = pool("ps_mm", 2, space="PSUM")
        ps_s = pool("ps_s", 2, space="PSUM")
        ps_y = pool("ps_y", 2, space="PSUM")
        ps_st = ps_y

        ones_sb = singles.tile([P, P], bf16)
        nc.vector.memset(ones_sb[:], 1.0)
        eps_sb = singles.tile([P, 1], f32)
        nc.vector.memset(eps_sb[:], EPS)

        gb = {}
        for name in ("g1", "b1", "g2", "b2"):
            t = singles.tile([P, NF], f32, name=f"gb_{name}")
            nc.sync.dma_start(out=t[:], in_=d[name].rearrange("(o p) -> p o", p=P))
            gb[name] = t

        mask_sb = singles.tile([P, 4, 4, P], bf16)
        nc.sync.dma_start(out=mask_sb[:], in_=d["mask4"])

        # ---------------- layer norm (feature-major) ----------------
        def layer_norm(src_bf_of, src_of, g_sb, b_sb, ncols, out_pool,
                       out_tag, out_dt):
            """src_bf_of(f, qt) -> [P,512] bf16 AP (stats pass);
            src_of(f, qt) -> [P,512] f32 AP (normalize pass).
            Returns NF tiles [P, ncols] of dtype out_dt."""
            outs = [out_pool.tile([P, ncols], out_dt, tag=out_tag,
                                  name=f"ln_{out_tag}_{i}") for i in range(NF)]
            for qt in range(ncols // 512):
                cs = slice(qt * 512, qt * 512 + 512)
                s1 = ps_st.tile([P, 512], f32, tag="y", name="s1")
                s2 = ps_st.tile([P, 512], f32, tag="y", name="s2")
                for f in range(NF):
                    xbf = src_bf_of(f, qt)
                    sq = bfp.tile([P, 512], bf16, tag="sq", name=f"sq{f}")
                    nc.vector.tensor_tensor(sq[:], xbf, xbf, ALU.mult)
                    nc.tensor.matmul(s1[:], ones_sb[:], xbf,
                                     start=(f == 0), stop=(f == NF - 1))
                    nc.tensor.matmul(s2[:], ones_sb[:], sq[:],
                                     start=(f == 0), stop=(f == NF - 1))
                mu = statp.tile([P, 512], f32, tag="stat", name="mu")
                nc.vector.tensor_scalar_mul(mu[:], s1[:], 1.0 / C)
                musq = statp.tile([P, 512], f32, tag="stat", name="musq")
                nc.scalar.activation(out=musq[:], in_=mu[:], func=AFT.Square)
                var = statp.tile([P, 512], f32, tag="stat", name="var")
                nc.vector.tensor_scalar(var[:], s2[:], 1.0 / C, None, ALU.mult)
                nc.vector.tensor_tensor(var[:], var[:], musq[:], ALU.subtract)
                std = statp.tile([P, 512], f32, tag="stat", name="std")
                nc.scalar.activation(out=std[:], in_=var[:], func=AFT.Sqrt,
                                     bias=eps_sb[:])
                rstd = statp.tile([P, 512], f32, tag="stat", name="rstd")
                nc.vector.reciprocal(rstd[:], std[:])
                for f in range(NF):
                    srcf = src_of(f, qt)
                    eng = nc.vector if f < 4 else nc.gpsimd
                    if unit_gb:
                        t = sqp.tile([P, 512], f32, tag="sq", name=f"lnt{f}")
                        eng.tensor_tensor(t[:], srcf, mu[:], ALU.subtract)
                        eng.tensor_tensor(outs[f][:, cs], t[:], rstd[:],
                                          ALU.mult)
                    else:
                        t = sqp.tile([P, 512], f32, tag="sq", name=f"lnt{f}")
                        nc.vector.tensor_tensor(t[:], srcf, mu[:], ALU.subtract)
                        nc.vector.tensor_tensor(t[:], t[:], rstd[:], ALU.mult)
                        nc.vector.tensor_scalar(outs[f][:, cs], t[:],
                                                g_sb[:, f:f + 1], b_sb[:, f:f + 1],
                                                ALU.mult, ALU.add)
            return outs

        # ---------------- LN1 (x streamed from DRAM, one load per chunk) ---
        x_cache = {}

        def x_src(f, qt):
            if (f, qt) not in x_cache:
                t = xstream2.tile([P, 512], f32, tag="x1", name=f"x_{f}_{qt}")
                if qt < NQT:
                    nc.sync.dma_start(out=t[:],
                                      in_=d["xT_r"][:, f, qt * 512:qt * 512 + 512])
                else:
                    nc.sync.dma_start(out=t[:], in_=d["xq_r"][:, f, :])
                x_cache[(f, qt)] = t
            return x_cache[(f, qt)][:]

        def x_bf_src(f, qt):
            t = bfp.tile([P, 512], bf16, tag="xbf", name=f"xbf_{f}_{qt}")
            nc.gpsimd.tensor_copy(out=t[:], in_=x_src(f, qt))
            return t[:]

        xln1 = layer_norm(x_bf_src, x_src, gb["g1"], gb["b1"], T + OWN, lnpool,
                          "ln1", bf16)

        # ---------------- V (token-major, + ones col per head) ----------------
        from contextlib import ExitStack as _ES
        attn_ctx = _ES()
        kpool = attn_ctx.enter_context(tc.tile_pool(name="kpool", bufs=2))
        qpool = attn_ctx.enter_context(tc.tile_pool(name="qpool", bufs=2))
        wv_p = attn_ctx.enter_context(tc.tile_pool(name="wv_p", bufs=1))
        apool = attn_ctx.enter_context(tc.tile_pool(name="apool", bufs=4))
        vpool = attn_ctx.enter_context(tc.tile_pool(name="vpool", bufs=1))
        v_sb = vpool.tile([P, NKT, 2, 6, HD + 1], bf16)
        nc.vector.memset(v_sb[:, :, :, :, HD], 1.0)
        for vn in range(2):
            w = wv_p.tile([P, NF, 384], bf16, tag="wv", name=f"wv{vn}")
            nc.sync.dma_start(out=w[:], in_=d["Wv_t"][vn])
            for kt in range(NKT):
                ps = ps_mm.tile([P, 512], f32, tag="mm", name="vps")
                ts = slice(kt * P, kt * P + P)
                for f in range(NF):
                    nc.tensor.matmul(ps[:, :384], xln1[f][:, ts], w[:, f, :],
                                     start=(f == 0), stop=(f == NF - 1))
                nc.any.tensor_copy(
                    out=v_sb[:, kt, vn, :, 0:HD],
                    in_=ps[:, :384].rearrange("p (j e) -> p j e", e=HD))

        # ---------------- per head-pair: Q, K, attention ----------------
        wap_sb = singles.tile([P, NF, C], bf16)
        nc.sync.dma_start(out=wap_sb[:], in_=d["WapR"])

        y_sb = [ypool.tile([P, OWN], bf16, tag="y", name=f"y_{i}")
                for i in range(NF)]
        for mo in range(NF):
            wq = wq_p.tile([P, NF, P], bf16, tag="wq", name=f"wq{mo}")
            nc.sync.dma_start(out=wq[:], in_=d["Wq_t"][mo])
            qt_sb = qpool.tile([P, OWN], bf16, tag="q", name=f"q{mo}")
            ps = ps_mm.tile([P, 512], f32, tag="mm", name="qps")
            for f in range(NF):
                nc.tensor.matmul(ps[:], wq[:, f, :], xln1[f][:, T:T + OWN],
                                 start=(f == 0), stop=(f == NF - 1))
            nc.any.tensor_copy(out=qt_sb[:], in_=ps[:])

            wk = wq_p.tile([P, NF, P], bf16, tag="wq", name=f"wk{mo}")
            nc.sync.dma_start(out=wk[:], in_=d["Wk_t"][mo])
            kt_sb = kpool.tile([P, T], bf16, tag="k", name=f"k{mo}")
            for qt in range(NQT):
                cs = slice(qt * 512, qt * 512 + 512)
                ps = ps_mm.tile([P, 512], f32, tag="mm", name="kps")
                for f in range(NF):
                    nc.tensor.matmul(ps[:], wk[:, f, :], xln1[f][:, cs],
                                     start=(f == 0), stop=(f == NF - 1))
                nc.any.tensor_copy(out=kt_sb[:, cs], in_=ps[:])

            for hh in (1, 0):
                h = 2 * mo + hh
                po = hh * HD
                yp = ps_y.tile([HD + 1, 512], f32, tag="y", name=f"yp{h}")
                for s in range(4):
                    BS = SLOT_BOUNDS[s]
                    q_s = qt_sb[po:po + HD, s * P:(s + 1) * P]
                    groups = [8] * (BS // 8) + ([4] if BS % 8 else [])
                    k0 = 0
                    for gi, gs in enumerate(groups):
                        sp = ps_s.tile([P, 8, P], f32, tag="s",
                                       name=f"sp{h}_{s}_{gi}")
                        for j in range(gs):
                            kt = k0 + j
                            nc.tensor.matmul(
                                sp[:, j, :],
                                kt_sb[po:po + HD, kt * P:(kt + 1) * P],
                                q_s, start=True, stop=True)
                        a_sb = apool.tile([P, 8, P], bf16, tag="a",
                                          name=f"a{h}_{s}_{gi}")
                        nc.scalar.activation(out=a_sb[:, :gs, :],
                                             in_=sp[:, :gs, :], func=AFT.Exp,
                                             scale=float(1.0 / np.sqrt(HD)))
                        if gi == len(groups) - 1:
                            nc.vector.tensor_tensor(a_sb[:, gs - 4:gs, :],
                                                    a_sb[:, gs - 4:gs, :],
                                                    mask_sb[:, s, :, :], ALU.mult)
                        for j in range(gs):
                            kt = k0 + j
                            nc.tensor.matmul(
                                yp[:, s * P:(s + 1) * P],
                                v_sb[:, kt, h // 6, h % 6, :], a_sb[:, j, :],
                                start=(kt == 0),
                                stop=(kt == BS - 1))
                        k0 += gs
                rec = recp.tile([1, 512], f32, tag="rec", name=f"rec{h}")
                nc.vector.reciprocal(rec[:], yp[HD:HD + 1, :])
                rd = dramp.tile([1, 512], f32, tag="rd", name=f"rd{h}")
                nc.gpsimd.dma_start(out=rd[:], in_=rec[:])
                rb = rbp.tile([HD, 512], f32, tag="rb", name=f"rb{h}")
                nc.gpsimd.dma_start(out=rb[:], in_=rd[:].broadcast_to([HD, 512]))
                if hh == 0:
                    nc.vector.tensor_tensor(y_sb[mo][0:HD, :], yp[0:HD, :],
                                            rb[:], ALU.mult)
                else:
                    yt = ytmp.tile([HD, 512], bf16, tag="yt", name=f"yt{h}")
                    nc.vector.tensor_tensor(yt[:], yp[0:HD, :], rb[:], ALU.mult)
                    nc.gpsimd.dma_start(out=y_sb[mo][HD:P, :], in_=yt[:])

        # ---------------- attn proj + residual ----------------
        x2 = []
        for mo in range(NF):
            # residual source: the LN1 qt=NQT stream tiles ARE xq -- still
            # resident in xstream2, so no re-load from DRAM is needed.
            xo = x_cache[(mo, NQT)]
            ps = ps_mm.tile([P, 512], f32, tag="mm", name="aps")
            for h2 in range(NF):
                nc.tensor.matmul(ps[:], wap_sb[:, h2, mo * P:(mo + 1) * P],
                                 y_sb[h2][:], start=(h2 == 0), stop=(h2 == NF - 1))
            x2t = x2pool.tile([P, OWN], f32, tag="x2", name=f"x2_{mo}")
            nc.vector.tensor_tensor(x2t[:], ps[:], xo[:], ALU.add)
            x2.append(x2t)

        attn_ctx.close()

        # ---------------- LN2 + MLP ----------------
        def x2_bf_src(f, qt):
            # DVE here: gpsimd is busy with late-attention broadcast DMAs,
            # while DVE is idle between attention and the MLP.
            t = bfp.tile([P, 512], bf16, tag="xbf", name=f"x2bf_{f}")
            nc.vector.tensor_copy(out=t[:], in_=x2[f][:])
            return t[:]

        xln2 = layer_norm(x2_bf_src, lambda f, qt: x2[f][:], gb["g2"], gb["b2"],
                          OWN, xlnp2, "ln2", bf16)

        mlp_ctx = _ES()
        wfcp = mlp_ctx.enter_context(tc.tile_pool(name="wfcp", bufs=2))
        wmp_p = mlp_ctx.enter_context(tc.tile_pool(name="wmp_p", bufs=2))
        hbig = mlp_ctx.enter_context(tc.tile_pool(name="hbig", bufs=1))
        h_sb = hbig.tile([P, NMO_FC, OWN], bf16)
        for g4 in range(NMO_FC // 4):
            w4 = wfcp.tile([P, 4, NF, P], bf16, tag="wfc", name=f"wfc{g4}")
            nc.sync.dma_start(out=w4[:], in_=d["Wfc_t"][g4])
            for i in range(4):
                mo = g4 * 4 + i
                ps = ps_mm.tile([P, 512], f32, tag="mm", name="fps")
                for f in range(NF):
                    nc.tensor.matmul(ps[:], w4[:, i, f, :], xln2[f][:],
                                     start=(f == 0), stop=(f == NF - 1))
                nc.scalar.activation(out=h_sb[:, mo, :], in_=ps[:],
                                     func=GELU_FUNC)
        for mo in range(NF):
            w = wmp_p.tile([P, NMO_FC, P], bf16, tag="wmp", name=f"wmp{mo}")
            nc.sync.dma_start(out=w[:], in_=d["Wmp_t"][mo])
            ps = ps_mm.tile([P, 512], f32, tag="mm", name="pps")
            for hc in range(NMO_FC):
                nc.tensor.matmul(ps[:], w[:, hc, :], h_sb[:, hc, :],
                                 start=(hc == 0), stop=(hc == NMO_FC - 1))
            ot = opool.tile([P, OWN], f32, tag="o", name=f"o{mo}")
            nc.vector.tensor_tensor(ot[:], ps[:], x2[mo][:], ALU.add)
            nc.gpsimd.dma_start(out=d["outT_r"][:, mo, :], in_=ot[:])
        mlp_ctx.close()


# ---------------------------------------------------------------------------
# host side
# ---------------------------------------------------------------------------

def make_core_inputs(inputs):
    """Build the 8 per-core input maps from the full-problem inputs."""
    import ml_dtypes

    x = np.asarray(inputs["x"], dtype=np.float32)
    bf = ml_dtypes.bfloat16

    def tile_w(w, n_mo, width):
        # [C_in, n_mo*width] -> [n_mo, 128, C_in//128, width]
        cin = w.shape[0]
        r = w.reshape(cin // P, P, n_mo, width)
        return np.ascontiguousarray(r.transpose(2, 1, 0, 3).astype(bf))

    Wqkv_f = np.asarray(inputs["W_qkv"], np.float32)
    Wfc_f = np.asarray(inputs["W_fc"], np.float32)
    Wmp_f = np.asarray(inputs["W_mlp_proj"], np.float32)
    wfc_t = tile_w(Wfc_f, 24, P).reshape(NF, 4, P, NF, P).transpose(0, 2, 1, 3, 4)
    full = {
        "Wq_t": tile_w(Wqkv_f[:, 0:C], NF, P),
        "Wk_t": tile_w(Wqkv_f[:, C:2 * C], NF, P),
        "Wv_t": tile_w(Wqkv_f[:, 2 * C:3 * C], 2, 384),
        "Wfc_t": np.ascontiguousarray(wfc_t),
        "Wmp_t": tile_w(Wmp_f, NF, P),
        "Wap": np.ascontiguousarray(np.asarray(inputs["W_attn_proj"], np.float32).astype(bf)),
        "g1": np.ascontiguousarray(np.asarray(inputs["ln1_g"], np.float32)),
        "b1": np.ascontiguousarray(np.asarray(inputs["ln1_b"], np.float32)),
        "g2": np.ascontiguousarray(np.asarray(inputs["ln2_g"], np.float32)),
        "b2": np.ascontiguousarray(np.asarray(inputs["ln2_b"], np.float32)),
    }
    in_maps = []
    for c in range(8):
        b, p = c // 4, c % 4
        blocks = [bs - 1 - p for bs in SLOT_BOUNDS]  # 128-row q-block indices
        xb = x[b]                                    # [T, C]
        own = np.concatenate([np.arange(bk * P, (bk + 1) * P) for bk in blocks])
        # mask4[kp, s, j, q] = keep for k-chunk (BS-4+j) of slot s vs q row
        mask = np.zeros((P, 4, 4, P), np.float32)
        kp = np.arange(P)[:, None]
        q = np.arange(P)[None, :]
        for s, BS in enumerate(SLOT_BOUNDS):
            E = BS - p          # real k-extent (chunks) for this core's block
            blk = BS - 1 - p    # q-block index
            for j in range(4):
                kc = BS - 4 + j
                if kc < E:
                    mask[:, s, j, :] = (kc * P + kp <= blk * P + q)
        m = dict(full)
        m["xT"] = np.ascontiguousarray(xb.T)
        m["xq"] = np.ascontiguousarray(xb[own].T)
        m["mask4"] = mask.astype(ml_dtypes.bfloat16)
        in_maps.append(m)
    return in_maps


def assemble_output(results):
    """results: list of 8 dicts with 'outT' [C, OWN] -> full [B, T, C] f32."""
    out = np.empty((B, T, C), dtype=np.float32)
    for c in range(8):
        b, p = c // 4, c % 4
        oT = results[c]["outT"].T  # [OWN, C] in slot order
        for s, BS in enumerate(SLOT_BOUNDS):
            blk = BS - 1 - p
            out[b, blk * P:(blk + 1) * P, :] = oT[s * P:(s + 1) * P, :]
    return out


_CACHED_NC = {}


def kernel(**inputs):
    from concourse.bass_utils import run_bass_kernel_spmd

    unit_gb = bool(
        np.all(np.asarray(inputs["ln1_g"]) == 1.0)
        and np.all(np.asarray(inputs["ln2_g"]) == 1.0)
        and np.all(np.asarray(inputs["ln1_b"]) == 0.0)
        and np.all(np.asarray(inputs["ln2_b"]) == 0.0))
    if unit_gb not in _CACHED_NC:
        _CACHED_NC[unit_gb] = build_program(unit_gb=unit_gb)
    in_maps = make_core_inputs(inputs)
    res = run_bass_kernel_spmd(_CACHED_NC[unit_gb], in_maps,
                               core_ids=list(range(8)))
    return assemble_output(res.results)


if __name__ == "__main__":
    nc = build_program()
    print("program built ok")

